# revision 38
# baseline (speedup 1.0000x reference)
"""Trainium2 Bass kernel for nn_AttentionBiasHead.

Per-sample attention with a post-softmax additive bias produced by an MLP whose
output Linear is huge (128 x 262144).  Strategy (8 NeuronCores):

- Data-parallel over batch: core i owns samples [4i, 4i+4).
- The bias-MLP output Linear (Wb2) is column-sharded BY l2 (key position):
  core i holds the columns for l2 in [64i, 64(i+1)) across ALL l1, ordered
  (l2, l1).  It computes those bias^T rows for all 32 samples; an fp16
  AllToAll delivers to each core the full bias^T (l2 on partitions) for its
  own 4 samples.
- Everything downstream is computed transposed so the post-A2A tail is just
  4 accumulating matmuls per sample:
    out^T[d, l1] = sum_j v_j @ P^T_j + sum_j v_j @ bb2T_j   (pre-A2A, PSUM
                   -> staged to SBUF)
                 + sum_j v_j @ biasT_j                      (post-A2A)
  where P^T = exp(scores^T * scale) normalized via a ones-matmul column sum
  and a K=1 broadcast matmul (no partition-dim reductions on DVE).
- qkv input loads are deferred behind the Wb2 stream so the bias pipeline
  owns early HBM bandwidth; the attention front overlaps the AllToAll.
"""

import numpy as np

N_CORES = 8
B, L, DIN, DQ, DS, DMLP = 32, 512, 512, 128, 256, 128
BPC = B // N_CORES          # samples per core = 4
NSH = L * L // N_CORES      # bias-shard columns per core = 32768
NT = NSH // 512             # Wb2 GEMM tiles (one tile = one l2 row) = 64
NQ = NT // 4                # GEMM quads (4 tiles stacked per PSUM bank) = 16
NGRP = 4                    # Wb2 DMA groups (16 tiles each, 2MB)
KT = DIN // 128             # contraction tiles for projections = 4
NC1 = L // 128              # 128-row chunks per sample = 4
SCALE = 1.0 / float(np.sqrt(DQ))

_cache = {}
DEBUG = False


def _build():
    if "nc" in _cache:
        return _cache["nc"]

    from contextlib import ExitStack

    import concourse.mybir as mybir
    import concourse.tile as tile
    from concourse import bacc
    from concourse.bass import ts, _add_dep_helper
    from concourse.masks import make_identity

    dt = mybir.dt
    f32, f16, u8 = dt.float32, dt.float16, dt.uint8

    nc = bacc.Bacc("TRN2", target_bir_lowering=False, debug=False,
                   num_devices=N_CORES)

    # ---- per-core external tensors -------------------------------------
    qT_d = nc.dram_tensor("qT", [BPC, 128, KT, L], f16, kind="ExternalInput").ap()
    kT_d = nc.dram_tensor("kT", [BPC, 128, KT, L], f16, kind="ExternalInput").ap()
    vT_d = nc.dram_tensor("vT", [BPC, 128, KT, L], f16, kind="ExternalInput").ap()
    mkT_d = nc.dram_tensor("mkT", [BPC, 128, NC1, L], u8, kind="ExternalInput").ap()
    HT_d = nc.dram_tensor("HT", [128, B], f16, kind="ExternalInput").ap()
    wqkv_d = nc.dram_tensor("wqkv", [128, KT, 3, DQ], f16, kind="ExternalInput").ap()
    bias4_d = nc.dram_tensor("bias4", [128, 4], f32, kind="ExternalInput").ap()
    Wb2s_d = nc.dram_tensor("Wb2s", [DMLP, NSH], f16, kind="ExternalInput").ap()
    bb2T_d = nc.dram_tensor("bb2T", [128, NC1, L], f16, kind="ExternalInput").ap()
    out_d = nc.dram_tensor("out", [BPC, L, DQ], f16, kind="ExternalOutput").ap()
    if DEBUG:
        dbg_exp_d = nc.dram_tensor("dbg_exp", [128, NC1, L], f16,
                                   kind="ExternalOutput").ap()
        dbg_opart_d = nc.dram_tensor("dbg_opart", [128, L], f32,
                                     kind="ExternalOutput").ap()
        dbg_biasT_d = nc.dram_tensor("dbg_biasT", [BPC, 128, NC1, L], f16,
                                     kind="ExternalOutput").ap()
        dbg_oT_d = nc.dram_tensor("dbg_oT", [BPC, 128, L], f16,
                                  kind="ExternalOutput").ap()
        dbg_a2ain_d = nc.dram_tensor("dbg_a2ain", [B, 2048], f16,
                                     kind="ExternalOutput").ap()

    with tile.TileContext(nc) as tc, ExitStack() as ctx:
        consts = ctx.enter_context(tc.tile_pool(name="consts", bufs=1))
        dram = ctx.enter_context(tc.tile_pool(name="dram", bufs=1, space="DRAM"))

        # ---- sync ring first: H^T (host-computed) then proj biases ------
        HT_sb = consts.tile([128, B], f16)
        nc.sync.dma_start(HT_sb[:], HT_d[:])
        bias4_sb = consts.tile([128, 4], f32)
        nc.sync.dma_start(bias4_sb[:], bias4_d[:])

        ident16 = consts.tile([128, 128], f16)
        make_identity(nc, ident16)
        ones_p = consts.tile([128, 1], f16)      # sumexp lhsT + masked value
        nc.vector.memset(ones_p, 1.0)
        ones_pp = consts.tile([128, 128], f16)   # sumexp lhsT (all ones)
        nc.vector.memset(ones_pp, 1.0)

        # ---- scalar ring: small attention consts ------------------------
        wqkv_sb = consts.tile([128, KT, 3, DQ], f16)
        nc.scalar.dma_start(wqkv_sb[:], wqkv_d[:])
        bb2T_sb = consts.tile([128, NC1, L], f16)
        nc.scalar.dma_start(bb2T_sb[:], bb2T_d[:])

        # ---- phase B: bias^T shard GEMM, split into two A2A halves -----
        a2a_in1 = dram.tile([B, NSH // 2], f16)
        a2a_out1 = dram.tile([B, NSH // 2], f16)
        a2a_in2 = dram.tile([B, NSH // 2], f16)
        a2a_out2 = dram.tile([B, NSH // 2], f16)
        # dst view per mega-write m: [s(32), t(16), 512]
        a2a_in_v = [
            a2a_in1.rearrange("s (m t w) -> m s t w", t=16, w=512),
            a2a_in2.rearrange("s (m t w) -> m s t w", t=16, w=512),
        ]

        inp = ctx.enter_context(tc.tile_pool(name="inp", bufs=BPC))
        mskp = ctx.enter_context(tc.tile_pool(name="mskp", bufs=BPC))
        qTin, kTin, vTin, mtile = {}, {}, {}, {}

        with tc.tile_pool(name="w2", bufs=2) as w2p, \
             tc.tile_pool(name="bsb", bufs=NGRP) as bsbp, \
             tc.tile_pool(name="bps", bufs=6, space="PSUM") as bpsp:
            w2ts = []
            a2a_wr = []
            for g in range(NGRP):
                w2t = w2p.tile([128, NT // NGRP, 512], f16, tag="w2t",
                               name=f"w2t{g}")
                w2d = nc.gpsimd.dma_start(
                    w2t[:], Wb2s_d[:, ts(g, (NT // NGRP) * 512)].rearrange(
                        "p (n w) -> p n w", w=512))
                if g > 0:
                    _add_dep_helper(w2d.ins, w2ts[g - 1][1].ins, sync=True,
                                    reason="chain Wb2 groups for arrival order")
                w2ts.append((w2t, w2d))
            gate = w2ts[NGRP - 1][1]

            cp_engines = [nc.scalar.copy, nc.vector.tensor_copy]
            NPM = NT // NGRP // 4           # quads per mega-write = 4
            bsb = None
            for u in range(NQ):
                g, tq = u // (NQ // NGRP), u % (NQ // NGRP)
                w2t = w2ts[g][0]
                if tq == 0:
                    bsb = bsbp.tile([32, 16, 512], f16, tag="bsb",
                                    name=f"bsb{g}")
                for t in range(4):
                    bq_ps = bpsp.tile([32, 512], f32, tag="bps",
                                      name=f"bps{u}_{t}")
                    nc.tensor.matmul(bq_ps[:], HT_sb[:],
                                     w2t[:, 4 * tq + t],
                                     start=True, stop=True)
                    cp_engines[(4 * u + t) % 2](
                        bsb[:, 4 * tq + t], bq_ps[:])
                if tq == NQ // NGRP - 1:
                    wd = nc.sync.dma_start(a2a_in_v[g // 2][g % 2], bsb[:])
                    a2a_wr.append(wd)

            # input loads: emitted after the writes so they can't block the
            # HWDGE rings; gated behind the last Wb2 group
            for s in range(BPC):
                qTin[s] = inp.tile([128, KT, L], f16, tag="qTin",
                                   name=f"qTin{s}")
                d = nc.scalar.dma_start(qTin[s][:], qT_d[s])
                if s == 0:
                    _add_dep_helper(d.ins, gate.ins, sync=True,
                                    reason="defer q behind Wb2 stream")
                kTin[s] = inp.tile([128, KT, L], f16, tag="kTin",
                                   name=f"kTin{s}")
                kd = nc.gpsimd.dma_start(kTin[s][:], kT_d[s])
                if s == 0:
                    _add_dep_helper(kd.ins, gate.ins, sync=True,
                                    reason="defer k behind Wb2 stream")
                vTin[s] = inp.tile([128, KT, L], f16, tag="vTin",
                                   name=f"vTin{s}")
                vd = nc.gpsimd.dma_start(vTin[s][:], vT_d[s])
                if s == 0:
                    _add_dep_helper(vd.ins, gate.ins, sync=True,
                                    reason="defer v behind Wb2 stream")
                mtile[s] = mskp.tile([128, NC1, L], u8, tag="mt",
                                     name=f"mt{s}")
                md = nc.gpsimd.dma_start(mtile[s][:], mkT_d[s])
                if s == 0:
                    _add_dep_helper(md.ins, gate.ins, sync=True,
                                    reason="defer mask behind Wb2 stream")

        # ---- collectives (gpsimd ring): half 1 fires at half-GEMM ------
        coll1 = nc.gpsimd.collective_compute(
            "AllToAll", mybir.AluOpType.bypass,
            replica_groups=[list(range(N_CORES))],
            ins=[a2a_in1.opt()], outs=[a2a_out1.opt()],
        )
        for wd in a2a_wr[:2]:
            _add_dep_helper(coll1.ins, wd.ins, sync=True,
                            reason="collective 1 waits half-1 writes")
        coll2 = nc.gpsimd.collective_compute(
            "AllToAll", mybir.AluOpType.bypass,
            replica_groups=[list(range(N_CORES))],
            ins=[a2a_in2.opt()], outs=[a2a_out2.opt()],
        )
        for wd in a2a_wr[2:]:
            _add_dep_helper(coll2.ins, wd.ins, sync=True,
                            reason="collective 2 waits half-2 writes")
        # a2a_outH rows: i*4 + s (i = source rank); cols (z(32), l1)
        a2a_v = [
            a2a_out1.rearrange("(i s) (z l) -> i s z l", s=BPC, l=L),
            a2a_out2.rearrange("(i s) (z l) -> i s z l", s=BPC, l=L),
        ]

        # ---- phase C: attention front (fully pre-A2A) ------------------
        prj = ctx.enter_context(tc.tile_pool(name="prj", bufs=2))
        rcp = ctx.enter_context(tc.tile_pool(name="rcp", bufs=2))
        vpool = ctx.enter_context(tc.tile_pool(name="vpool", bufs=BPC))
        expp = ctx.enter_context(tc.tile_pool(name="expp", bufs=BPC * NC1))
        opart = ctx.enter_context(tc.tile_pool(name="opart", bufs=BPC))
        cps = ctx.enter_context(tc.tile_pool(name="cps", bufs=1, space="PSUM"))
        exp_t, v_t, opart_t = {}, {}, {}

        # stage-major emission: every engine ring flows without
        # cross-sample round-trip stalls
        qT_t, kT_t, vT_t, se_t, rb_t, ops_t = {}, {}, {}, {}, {}, {}
        scalar_tail_deps = []
        for s in range(BPC):
            q_ps = cps.tile([128, 512], f32, tag="pp", bufs=2, name=f"qps{s}")
            for kt in range(KT):
                nc.tensor.matmul(q_ps[:], wqkv_sb[:, kt, 0], qTin[s][:, kt],
                                 start=(kt == 0), stop=(kt == KT - 1))
            qT_sb = prj.tile([128, L], f16, tag="qT", name=f"qT{s}")
            nc.scalar.activation(qT_sb[:], q_ps[:],
                                 mybir.ActivationFunctionType.Identity,
                                 bias=bias4_sb[:, 0:1], scale=1.0)
            qT_t[s] = qT_sb

            k_ps = cps.tile([128, 512], f32, tag="pp", bufs=2, name=f"kps{s}")
            for kt in range(KT):
                nc.tensor.matmul(k_ps[:], wqkv_sb[:, kt, 1], kTin[s][:, kt],
                                 start=(kt == 0), stop=(kt == KT - 1))
            kT_sb = prj.tile([128, L], f16, tag="kT", name=f"kT{s}")
            nc.scalar.activation(kT_sb[:], k_ps[:],
                                 mybir.ActivationFunctionType.Identity,
                                 bias=bias4_sb[:, 1:2], scale=1.0)
            kT_t[s] = kT_sb

            w_ps = cps.tile([128, 512], f32, tag="pp", bufs=2, name=f"wps{s}")
            for kt in range(KT):
                nc.tensor.matmul(w_ps[:], wqkv_sb[:, kt, 2], vTin[s][:, kt],
                                 start=(kt == 0), stop=(kt == KT - 1))
            vT_sb = prj.tile([128, L], f16, tag="vTs", name=f"vTs{s}")
            nc.scalar.activation(vT_sb[:], w_ps[:],
                                 mybir.ActivationFunctionType.Identity,
                                 bias=bias4_sb[:, 2:3], scale=1.0)

            # scores^T chunks (l2 on partitions), exp, mask -> 1.0
            for c in range(NC1):
                sc_ps = cps.tile([128, 512], f32, tag="sc", bufs=2,
                                 name=f"sc{s}_{c}")
                nc.tensor.matmul(sc_ps[:], kT_sb[:, ts(c, 128)], qT_sb[:],
                                 start=True, stop=True)
                exp_sb = expp.tile([128, L], f16, tag="exp", name=f"ex{s}_{c}")
                exp_ins = nc.scalar.activation(
                    exp_sb[:], sc_ps[:], mybir.ActivationFunctionType.Exp,
                    bias=0.0, scale=SCALE)
                nc.vector.copy_predicated(exp_sb[:], mtile[s][:, c],
                                          ones_p[:].to_broadcast([128, 512]))
                exp_t[(s, c)] = exp_sb

            # v^T -> v (l2 on partitions) while exp/mask drain
            v_ps = cps.tile([128, 512], f16, tag="vtp", bufs=1,
                            padded_shape=[128, 1024], name=f"vps{s}")
            for j in range(NC1):
                nc.tensor.transpose(v_ps[:, ts(j, 128)], vT_sb[:, ts(j, 128)],
                                    ident16)
            v_sb = vpool.tile([128, NC1, DQ], f16, tag="v", name=f"v{s}")
            nc.vector.tensor_copy(
                v_sb[:], v_ps[:].rearrange("p (j d) -> p j d", j=NC1))
            v_t[s] = v_sb

        for s in range(BPC):
            # sumexp over l2 via all-ones matmul, wide fast reciprocal
            se_ps = cps.tile([128, 512], f32, tag="srb", bufs=1,
                             name=f"se{s}")
            for c in range(NC1):
                nc.tensor.matmul(se_ps[:], ones_pp[:], exp_t[(s, c)][:],
                                 start=(c == 0), stop=(c == NC1 - 1))
            recipB = rcp.tile([128, 512], f32, tag="rc", name=f"rc{s}")
            nc.vector.reciprocal_approx_fast(recipB[:], se_ps[:])
            for c in range(NC1):
                nc.vector.tensor_tensor(exp_t[(s, c)][:], exp_t[(s, c)][:],
                                        recipB[:], mybir.AluOpType.mult)
        for s in range(BPC):
            # out^T partial: P@v + bb2@v, staged to SBUF to free the bank
            o_ps = cps.tile([128, 512], f32, tag="op", bufs=2, name=f"oT{s}")
            for j in range(NC1):
                nc.tensor.matmul(o_ps[:], v_t[s][:, j], exp_t[(s, j)][:],
                                 start=(j == 0), stop=False)
            for j in range(NC1):
                nc.tensor.matmul(o_ps[:], v_t[s][:, j], bb2T_sb[:, j],
                                 start=False, stop=(j == NC1 - 1))
            ops_t[s] = o_ps
        for s in range(BPC):
            op_sb = opart.tile([128, 512], f32, tag="opart", name=f"opart{s}")
            if s % 2 == 0:
                scalar_tail_deps.append(nc.scalar.copy(op_sb[:], ops_t[s][:]))
            else:
                nc.vector.tensor_copy(op_sb[:], ops_t[s][:])
            opart_t[s] = op_sb

        scalar_tail_deps.append(exp_ins)

        # ---- phase D: post-A2A tail, one stage per collective half -----
        bi = ctx.enter_context(tc.tile_pool(name="bi", bufs=BPC))
        outp = ctx.enter_context(tc.tile_pool(name="outp", bufs=2))
        halves = [(coll1, [nc.sync, nc.scalar]),
                  (coll2, [nc.sync, nc.scalar, nc.gpsimd])]
        biasT_t, bps_t = {}, {}
        for s in range(BPC):
            biasT_t[s] = bi.tile([128, NC1, L], f16, tag="biasT",
                                 name=f"bT{s}")
            bps_t[s] = cps.tile([128, 512], f32, tag="op", bufs=2,
                                name=f"bps_{s}")
        for h, (coll, lde) in enumerate(halves):
            for s in range(BPC):
                biasT = biasT_t[s]
                for i in range(8):
                    eng = lde[i % len(lde)]
                    ld = eng.dma_start(
                        biasT[ts(i % 4, 32), 2 * h + i // 4],
                        a2a_v[h][i, s])
                    _add_dep_helper(ld.ins, coll.ins, sync=True,
                                    reason="biasT load waits collective")
                    if eng is nc.scalar:
                        for dp in scalar_tail_deps:
                            _add_dep_helper(ld.ins, dp.ins, sync=True,
                                            reason="keep load after phase C")
                for c in (2 * h, 2 * h + 1):
                    nc.tensor.matmul(bps_t[s][:], v_t[s][:, c],
                                     biasT[:, c],
                                     start=(c == 0), stop=(c == NC1 - 1),
                                     skip_group_check=True)
        for s in range(BPC):
            oT_sb = outp.tile([128, L], f16, tag="oT", name=f"oTs{s}")
            nc.vector.tensor_tensor(oT_sb[:], bps_t[s][:], opart_t[s][:],
                                    mybir.AluOpType.add)
            o_sb = outp.tile([128, NC1, DQ], f16, tag="o", name=f"os{s}")
            oeng = nc.sync if s % 2 == 0 else nc.scalar
            oeng.dma_start_transpose(o_sb[:], oT_sb[:])
            oeng.dma_start(out_d[s].rearrange("(j p) d -> p j d", p=128),
                           o_sb[:])
            if DEBUG:
                nc.scalar.dma_start(dbg_biasT_d[s], biasT_t[s][:])
                nc.scalar.dma_start(dbg_oT_d[s], oT_sb[:])
                if s == 0:
                    for c in range(NC1):
                        nc.scalar.dma_start(dbg_exp_d[:, c], exp_t[(0, c)][:])
                    nc.scalar.dma_start(dbg_opart_d[:], opart_t[0][:])

    nc.compile()
    _cache["nc"] = nc
    return nc


def _prep_in_maps(query, key, value, sf, atten_mask, Wq, bq, Wk, bk, Wv, bv,
                  Wb1, bb1, Wb2, bb2):
    f16 = np.float16
    HT = np.ascontiguousarray(np.maximum(
        np.asarray(sf, np.float32) @ np.asarray(Wb1, np.float32)
        + np.asarray(bb1, np.float32), 0.0).T.astype(f16))
    wqkv = np.ascontiguousarray(
        np.stack([np.asarray(Wq, f16), np.asarray(Wk, f16),
                  np.asarray(Wv, f16)], axis=1)
        .reshape(KT, 128, 3, DQ).transpose(1, 0, 2, 3))
    bias4 = np.ascontiguousarray(
        np.stack([np.asarray(bq, np.float32), np.asarray(bk, np.float32),
                  np.asarray(bv, np.float32), np.asarray(bb1, np.float32)],
                 axis=1))
    # bias^T chunks: bb2T[p, c, l1] = bb2[l1 * L + (c*128 + p)]
    bb2T = np.ascontiguousarray(
        np.asarray(bb2, f16).reshape(L, L).T.reshape(NC1, 128, L)
        .transpose(1, 0, 2))
    # Wb2 shard for core i: columns (l2, l1) for
    # l2 in [32i, 32i+32) U [256+32i, 256+32i+32), ordered l2-major
    Wb2_16 = np.asarray(Wb2, f16).reshape(DMLP, L, L)  # [m, l1, l2]

    def tr_in(x):
        # [4, l, din] -> [4, p(128), kt, l]
        xt = np.asarray(x, f16).transpose(0, 2, 1)
        return np.ascontiguousarray(
            xt.reshape(BPC, KT, 128, L).transpose(0, 2, 1, 3))

    in_maps = []
    for i in range(N_CORES):
        sl = slice(BPC * i, BPC * (i + 1))
        # mask^T: mkT[s][p, c, l1] = mask[s][l1, l2 = c*128 + p]
        mkT = np.asarray(atten_mask[sl], np.uint8).transpose(0, 2, 1)
        w2s = np.concatenate(
            [Wb2_16[:, :, 32 * i: 32 * i + 32],
             Wb2_16[:, :, 256 + 32 * i: 256 + 32 * i + 32]],
            axis=2).transpose(0, 2, 1)
        in_maps.append({
            "qT": tr_in(query[sl]),
            "kT": tr_in(key[sl]),
            "vT": tr_in(value[sl]),
            "mkT": np.ascontiguousarray(
                mkT.reshape(BPC, NC1, 128, L).transpose(0, 2, 1, 3)),
            "HT": HT,
            "wqkv": wqkv,
            "bias4": bias4,
            "Wb2s": np.ascontiguousarray(w2s.reshape(DMLP, NSH)),
            "bb2T": bb2T,
        })
    return in_maps


def kernel(**inputs) -> np.ndarray:
    from concourse import bass_utils
    nc = _build()
    in_maps = _prep_in_maps(**inputs)
    res = bass_utils.run_bass_kernel_spmd(
        nc, in_maps, core_ids=list(range(N_CORES)))
    return np.concatenate([r["out"] for r in res.results],
                          axis=0).astype(np.float32)
